# revision 4
# baseline (speedup 1.0000x reference)
"""Trainium2 Bass kernel for nn_ComplexityAttention (mu-gated GQA attention).

Distribution: 8 NeuronCores = 2 batches x 4 kv-groups. Core c handles
batch b=c//4 and kv-group g=c%4 (4 q-heads + 1 kv-head). No collectives:
each core emits a partial output [N, D] (its head-group's contribution
through the row-sharded wo), and the host sums the 4 partials per batch.

Device pipeline per core:
  A) QKV projection (bf16 matmuls, f32 PSUM accum over 4096-dim concat
     [x, mu_prev] features) -> per-head RMSNorm (ACT square+accum,
     sqrt, DVE reciprocal) -> RoPE (DVE, tables with norm-weights folded
     in; even/odd dims pre-permuted on host so RoPE is contiguous)
     -> PE transpose of Q,K into [hd, token] layout.
  B) Attention per (head, query-block of 512): S^T = K @ Q^T on PE,
     exp on ACT (no max-subtraction; scores are bounded), P@V and
     ones-reduction (softmax denominator) accumulated on PE,
     reciprocal + normalize on DVE, gpsimd partition-broadcast.
  C) Output projection out^T_h @ wo_h^T accumulated over 4 heads.

Host does all layout work: transposes, tiling, bf16 conversion,
even/odd permutation of wq/wk rows, folding q/k norm weights into
cos/sin tables, and the final sum over kv-groups.
"""

import os
import sys
import types
from contextlib import ExitStack

import numpy as np

for _p in ("/opt/trn_rl_repo", "/root/.axon_site/_ro/trn_rl_repo"):
    if os.path.isdir(_p) and _p not in sys.path:
        sys.path.append(_p)

import ml_dtypes  # noqa: E402
import concourse.bass as bass  # noqa: E402
import concourse.tile as tile  # noqa: E402
from concourse import bacc, mybir  # noqa: E402
from concourse.bass_utils import run_bass_kernel_spmd  # noqa: E402
from concourse.masks import make_identity  # noqa: E402

BF16 = mybir.dt.bfloat16
F32 = mybir.dt.float32
AF = mybir.ActivationFunctionType

B, N, D = 2, 2048, 2048
H, KVH = 16, 4
HD = 128                 # head dim
REP = H // KVH           # q heads per kv group (= per core)
QDIM = REP * HD          # 512 q dims per core
F2 = 2 * D               # 4096 concatenated feature dim
KT = F2 // 128           # 32 contraction tiles
MT = N // 128            # 16 token tiles
NQB = 4                  # query blocks of 512
EPS = 1e-6
SCALE = float(HD) ** -0.5
NCORES = 8

_nc_cache = None


def _kernel_body(tc, xmu, wqkv, wo, tabs, out):
    nc = tc.nc
    with ExitStack() as ctx:
        singles = ctx.enter_context(tc.tile_pool(name="singles", bufs=1))
        xpool = ctx.enter_context(tc.tile_pool(name="xpool", bufs=2))
        tpool = ctx.enter_context(tc.tile_pool(name="tpool", bufs=2))
        wpool = ctx.enter_context(tc.tile_pool(name="wpool", bufs=2))
        ppool = ctx.enter_context(tc.tile_pool(name="ppool", bufs=20))
        apool = ctx.enter_context(tc.tile_pool(name="apool", bufs=2))
        opool = ctx.enter_context(tc.tile_pool(name="opool", bufs=3))
        psum = ctx.enter_context(tc.tile_pool(name="psum", bufs=8, space="PSUM"))

        wqkv_sb = singles.tile([128, KT, 768], BF16)
        nc.sync.dma_start(wqkv_sb, wqkv)
        wo_sb = singles.tile([128, REP, D], BF16)
        nc.sync.dma_start(wo_sb, wo)
        ident = singles.tile([128, 128], BF16)
        make_identity(nc, ident)
        ones_sb = singles.tile([128, 1], BF16)
        nc.vector.memset(ones_sb, 1.0)
        eps_sb = singles.tile([128, 1], F32)
        nc.vector.memset(eps_sb, EPS)
        QT = singles.tile([128, REP, N], BF16)   # q^T per head: [hd, tok]
        KTr = singles.tile([128, N], BF16)       # k^T: [hd, tok]
        Vt = singles.tile([128, MT, HD], BF16)   # v: [tok(128/tile), kt, hd]
        OT = singles.tile([128, REP, N], BF16)   # attn out^T per head

        # ---- Phase A: QKV + RMSNorm + RoPE + transpose ----
        for m in range(MT):
            xt = xpool.tile([128, KT, 128], BF16, tag="xt")
            nc.sync.dma_start(xt, xmu[m])
            qtab = tpool.tile([128, 16, 64], BF16, tag="qtab")
            nc.sync.dma_start(qtab, tabs[m][:, 0:16])
            ktab = tpool.tile([128, 4, 64], BF16, tag="ktab")
            nc.sync.dma_start(ktab, tabs[m][:, 16:20])

            psq = psum.tile([128, QDIM], F32, tag="ps", name="psq")
            pskv = psum.tile([128, 256], F32, tag="ps", name="pskv")
            for k in range(KT):
                nc.tensor.matmul(psq, xt[:, k], wqkv_sb[:, k, 0:QDIM],
                                 start=(k == 0), stop=(k == KT - 1))
                nc.tensor.matmul(pskv, xt[:, k], wqkv_sb[:, k, QDIM:768],
                                 start=(k == 0), stop=(k == KT - 1))

            # RMSNorm q (per head) and k
            qn = wpool.tile([128, REP, HD], BF16, tag="qn")
            for h in range(REP):
                sq = wpool.tile([128, HD], F32, tag="sq")
                ssq = wpool.tile([128, 1], F32, tag="ssq")
                nc.scalar.activation(out=sq, in_=psq[:, h * HD:(h + 1) * HD],
                                     func=AF.Square, accum_out=ssq)
                std = wpool.tile([128, 1], F32, tag="std")
                nc.scalar.activation(out=std, in_=ssq, func=AF.Sqrt,
                                     bias=eps_sb, scale=1.0 / HD)
                r = wpool.tile([128, 1], F32, tag="r")
                nc.vector.reciprocal(r, std)
                nc.vector.tensor_scalar_mul(qn[:, h],
                                            psq[:, h * HD:(h + 1) * HD], r)
            kn = wpool.tile([128, HD], BF16, tag="kn")
            sqk = wpool.tile([128, HD], F32, tag="sqk")
            ssqk = wpool.tile([128, 1], F32, tag="ssqk")
            nc.scalar.activation(out=sqk, in_=pskv[:, 0:HD],
                                 func=AF.Square, accum_out=ssqk)
            stdk = wpool.tile([128, 1], F32, tag="stdk")
            nc.scalar.activation(out=stdk, in_=ssqk, func=AF.Sqrt,
                                 bias=eps_sb, scale=1.0 / HD)
            rk = wpool.tile([128, 1], F32, tag="rk")
            nc.vector.reciprocal(rk, stdk)
            nc.vector.tensor_scalar_mul(kn, pskv[:, 0:HD], rk)

            # V: plain copy PSUM -> SBUF (bf16 cast)
            nc.scalar.activation(out=Vt[:, m], in_=pskv[:, HD:256], func=AF.Copy)

            # RoPE q (batched over heads; tables already head-replicated)
            qa = wpool.tile([128, REP, 64], BF16, tag="qa")
            qb2 = wpool.tile([128, REP, 64], BF16, tag="qb2")
            qr = wpool.tile([128, REP, HD], BF16, tag="qr")
            nc.vector.tensor_mul(qa, qn[:, :, 0:64], qtab[:, 0:4])
            nc.vector.tensor_mul(qb2, qn[:, :, 64:128], qtab[:, 4:8])
            nc.vector.tensor_sub(qr[:, :, 0:64], qa, qb2)
            qc = wpool.tile([128, REP, 64], BF16, tag="qc")
            qd = wpool.tile([128, REP, 64], BF16, tag="qd")
            nc.vector.tensor_mul(qc, qn[:, :, 0:64], qtab[:, 8:12])
            nc.vector.tensor_mul(qd, qn[:, :, 64:128], qtab[:, 12:16])
            nc.vector.tensor_add(qr[:, :, 64:128], qc, qd)

            # RoPE k
            ka = wpool.tile([128, 64], BF16, tag="ka")
            kb = wpool.tile([128, 64], BF16, tag="kb")
            kr = wpool.tile([128, HD], BF16, tag="kr")
            nc.vector.tensor_mul(ka, kn[:, 0:64], ktab[:, 0])
            nc.vector.tensor_mul(kb, kn[:, 64:128], ktab[:, 1])
            nc.vector.tensor_sub(kr[:, 0:64], ka, kb)
            kc = wpool.tile([128, 64], BF16, tag="kc")
            kd = wpool.tile([128, 64], BF16, tag="kd")
            nc.vector.tensor_mul(kc, kn[:, 0:64], ktab[:, 2])
            nc.vector.tensor_mul(kd, kn[:, 64:128], ktab[:, 3])
            nc.vector.tensor_add(kr[:, 64:128], kc, kd)

            # Transpose to [hd, tok]
            for h in range(REP):
                ptr = psum.tile([128, 128], BF16, tag="ps", name="ptr")
                nc.tensor.transpose(ptr, qr[:, h], ident)
                nc.scalar.activation(out=QT[:, h, m * 128:(m + 1) * 128],
                                     in_=ptr, func=AF.Copy)
            ptrk = psum.tile([128, 128], BF16, tag="ps", name="ptrk")
            nc.tensor.transpose(ptrk, kr, ident)
            nc.scalar.activation(out=KTr[:, m * 128:(m + 1) * 128],
                                 in_=ptrk, func=AF.Copy)

        # ---- Phase B: attention ----
        for h in range(REP):
            for qb in range(NQB):
                qs = slice(qb * 512, (qb + 1) * 512)
                po = psum.tile([128, 512], F32, tag="ps", name="po")
                pd = psum.tile([1, 512], F32, tag="ps", name="pd")
                pts = []
                for k in range(MT):
                    ps_s = psum.tile([128, 512], F32, tag="ps", name="ps_s")
                    nc.tensor.matmul(ps_s, KTr[:, k * 128:(k + 1) * 128],
                                     QT[:, h, qs], start=True, stop=True)
                    pt = ppool.tile([128, 512], BF16, tag="pt")
                    nc.scalar.activation(out=pt, in_=ps_s, func=AF.Exp,
                                         scale=SCALE)
                    pts.append(pt)
                for k in range(MT):
                    nc.tensor.matmul(po, Vt[:, k], pts[k],
                                     start=(k == 0), stop=(k == MT - 1))
                for k in range(MT):
                    nc.tensor.matmul(pd, ones_sb, pts[k],
                                     start=(k == 0), stop=(k == MT - 1))
                rec = apool.tile([1, 512], F32, tag="rec")
                nc.vector.reciprocal_approx_fast(out=rec, in_=pd)
                rb = apool.tile([128, 512], F32, tag="rb")
                nc.gpsimd.partition_broadcast(rb, rec)
                nc.vector.tensor_mul(OT[:, h, qs], po, rb)

        # ---- Phase C: output projection (partial over this core's heads) ----
        for m in range(MT):
            for ob in range(4):
                pp = psum.tile([128, 512], F32, tag="ps", name="pp")
                for h in range(REP):
                    nc.tensor.matmul(pp, OT[:, h, m * 128:(m + 1) * 128],
                                     wo_sb[:, h, ob * 512:(ob + 1) * 512],
                                     start=(h == 0), stop=(h == REP - 1))
                osb = opool.tile([128, 512], F32, tag="osb")
                nc.scalar.activation(out=osb, in_=pp, func=AF.Copy)
                nc.sync.dma_start(
                    out[m * 128:(m + 1) * 128, ob * 512:(ob + 1) * 512], osb)


def _build_nc():
    nc = bacc.Bacc("TRN2", target_bir_lowering=False, debug=False,
                   num_devices=NCORES)
    xmu = nc.dram_tensor("xmu", [MT, 128, KT, 128], BF16,
                         kind="ExternalInput").ap()
    wqkv = nc.dram_tensor("wqkv", [128, KT, 768], BF16,
                          kind="ExternalInput").ap()
    wo = nc.dram_tensor("wo", [128, REP, D], BF16, kind="ExternalInput").ap()
    tabs = nc.dram_tensor("tabs", [MT, 128, 20, 64], BF16,
                          kind="ExternalInput").ap()
    out = nc.dram_tensor("out", [N, D], F32, kind="ExternalOutput").ap()
    with tile.TileContext(nc) as tc:
        _kernel_body(tc, xmu, wqkv, wo, tabs, out)
    nc.compile()
    return nc


_PERM = np.concatenate([np.arange(0, HD, 2), np.arange(1, HD, 2)])


def _bf16(a):
    return np.ascontiguousarray(a).astype(ml_dtypes.bfloat16)


def _prep_shared(inputs):
    """Host prep that doesn't depend on the core: tables + per-(b,g) arrays."""
    x = np.asarray(inputs["x"], np.float32)
    mu = np.asarray(inputs["mu_prev"], np.float32)
    cos = np.asarray(inputs["cos"], np.float32)
    sin = np.asarray(inputs["sin"], np.float32)
    qnw = np.asarray(inputs["q_norm_w"], np.float32)
    knw = np.asarray(inputs["k_norm_w"], np.float32)
    wq = np.asarray(inputs["wq"], np.float32)
    wk = np.asarray(inputs["wk"], np.float32)
    wv = np.asarray(inputs["wv"], np.float32)
    mqw = np.asarray(inputs["mu_q_w"], np.float32)
    mkw = np.asarray(inputs["mu_k_w"], np.float32)
    mvw = np.asarray(inputs["mu_v_w"], np.float32)
    wo = np.asarray(inputs["wo"], np.float32)

    # RoPE tables with norm weights folded in (permuted even/odd space):
    # out1 = t1*C1 - t2*S2 ; out2 = t1*S1 + t2*C2
    we, wo_ = qnw[0::2], qnw[1::2]
    qparts = [cos * we, sin * wo_, sin * we, cos * wo_]
    we_k, wo_k = knw[0::2], knw[1::2]
    kparts = [cos * we_k, sin * wo_k, sin * we_k, cos * wo_k]
    tab_list = [qparts[j] for j in range(4) for _ in range(REP)] + kparts
    tabs = np.stack(tab_list, axis=1)                # [N, 20, 64]
    tabs_arr = _bf16(tabs.reshape(MT, 128, 20, 64))

    # Per-batch xmu, pre-tiled [m, f, kt, t]
    xmu_arrs = []
    for b in range(B):
        xm = np.concatenate([x[b], mu[b]], axis=1)   # [N, 4096]
        xm = xm.reshape(MT, 128, KT, 128).transpose(0, 3, 2, 1)
        xmu_arrs.append(_bf16(xm))

    # Per-group weights
    wqkv_arrs, wo_arrs = [], []
    for g in range(KVH):
        qs = slice(g * QDIM, (g + 1) * QDIM)
        kvs = slice(g * HD, (g + 1) * HD)
        perm_q = np.concatenate([hh * HD + _PERM for hh in range(REP)])
        Wq = np.concatenate([wq[qs], mqw[qs]], axis=1)[perm_q]   # [512, 4096]
        Wk = np.concatenate([wk[kvs], mkw[kvs]], axis=1)[_PERM]  # [128, 4096]
        Wv = np.concatenate([wv[kvs], mvw[kvs]], axis=1)         # [128, 4096]
        Wg = np.concatenate([Wq, Wk, Wv], axis=0)                # [768, 4096]
        wqkv_arrs.append(_bf16(
            Wg.T.reshape(KT, 128, 768).transpose(1, 0, 2)))
        wo_g = wo[:, g * QDIM:(g + 1) * QDIM].T                  # [512, D]
        wo_arrs.append(_bf16(
            wo_g.reshape(REP, HD, D).transpose(1, 0, 2)))

    in_maps = []
    for c in range(NCORES):
        b, g = divmod(c, KVH)
        in_maps.append({
            "xmu": xmu_arrs[b],
            "wqkv": wqkv_arrs[g],
            "wo": wo_arrs[g],
            "tabs": tabs_arr,
        })
    return in_maps


def _install_ntff_hook():
    try:
        import antenv.axon_hooks as m
        if m.get_axon_ntff_profile_hook() is not None:
            return True
    except ImportError:
        import antenv
        m = types.ModuleType("antenv.axon_hooks")
        m._hook = None
        m.set_axon_ntff_profile_hook = lambda h: setattr(m, "_hook", h)
        m.get_axon_ntff_profile_hook = lambda: m._hook
        sys.modules["antenv.axon_hooks"] = m
        antenv.axon_hooks = m
    try:
        from trn_agent_boot.trn_boot import _ntff_profile_via_ctypes
        m.set_axon_ntff_profile_hook(
            _ntff_profile_via_ctypes("/opt/axon/libaxon_pjrt.so"))
    except Exception:
        return False
    return m.get_axon_ntff_profile_hook() is not None


def run(inputs, trace=False, tmpdir=None):
    """Returns (output [B,N,D] f32, BassKernelResults)."""
    global _nc_cache
    if trace:
        _install_ntff_hook()
    if _nc_cache is None:
        _nc_cache = _build_nc()
    in_maps = _prep_shared(inputs)
    res = run_bass_kernel_spmd(_nc_cache, in_maps,
                               core_ids=list(range(NCORES)),
                               trace=trace, tmpdir=tmpdir)
    parts = np.stack([np.asarray(res.results[c]["out"], np.float32)
                      for c in range(NCORES)])
    outv = parts.reshape(B, KVH, N, D).sum(axis=1).astype(np.float32)
    return outv, res


def kernel(**inputs):
    outv, _ = run(inputs, trace=False)
    return outv
